# revision 2
# baseline (speedup 1.0000x reference)
"""Trainium2 Bass kernel for ConditionalAttentionFusion-v2 (v2 redesign).

Math (per batch b, channel c, pixel y,x):
    G    = a0*rgb + a1*d + conv3x3(vars; Wc[c])          a0=Wt0*Wp0, a1=Wt0*Wp1,
                                                          Wc = Wt1*W_unc
    out  = rgb*G + d*(1-G) = rgb - diff*Q,  diff = rgb-d
    Q    = 1 - G = 1 - s*rgb + a1*diff - conv,  s = a0+a1

Strategy: pure data parallel over 8 cores.  The 4 images x 512 rows are cut
into 344 global row-groups of 6 rows (86 per image, 4 pad rows at each image
bottom); each core owns 43 consecutive groups, processed 4 at a time
(10 iters x 4096 cols + 1 iter x 3072).  All I/O is bf16 (gate is 2e-2,
measured ~9e-3), partition dim is the 114 real (c,yl) pairs - no padding.

Per-core HBM traffic ~31.5 MB (was 37.8): one combined rgb|diff load
[114, 2*ng*1024] (1.87 MB, sync/HWDGE queue) and one out store (0.93 MB,
scalar/HWDGE queue) per iter, plus vars loaded once, compactly (1.46 MB
total): 16 partitions per group = 2 maps x 8 halo rows, x-halo included
(1032 cols/group).  The 3 kx-shifted replicas the conv matmul needs are
built on-chip per iter by strided SBUF->SBUF DMA (no HBM cost).

Per 512-col PSUM block the PE accumulates the WHOLE linear part of Q:
    ps = -conv (b48, K=48, contraction q=(kx, i, halo-row))
         - diag(s) @ rgb  - (-diag(a1)) @ diff          (two K=114 diagonals)
ACT drains PSUM with the fused "+1" bias (tq = ps + 1 = Q), leaving DVE a
2-op bf16 tail: out = rgb - diff*Q.  Engine budget per 8.2us iter: DMA 8.2,
PE ~5.1 (warm), ACT ~4.4, DVE ~3.8, GpSimd ~2 (SWDGE issue).
"""
import sys

if "/opt/trn_rl_repo" not in sys.path:
    sys.path.insert(0, "/opt/trn_rl_repo")

import numpy as np

import concourse.bacc as bacc
import concourse.mybir as mybir
import concourse.tile as tile
from concourse.bass_utils import run_bass_kernel_spmd

F32 = mybir.dt.float32
BF16 = mybir.dt.bfloat16
NPBF = mybir.dt.np(BF16)

B, C, H, W = 4, 19, 512, 1024
YL = 6                  # rows per group
GPI = 86                # groups per image (516 rows, 4 pad)
NGRP = B * GPI          # 344 global groups
NCORES = 8
NG = NGRP // NCORES     # 43 groups per core
M = C * YL              # 114 output partitions
K = 48                  # conv contraction: 3 kx * 2 maps * 8 halo rows
VW = W + 8              # 1032 var cols per group (1 + 1024 + 1 halo, pad 6)
ITERS = [(4 * i, 4) for i in range(10)] + [(40, 3)]   # (group0, ngroups)


# ----------------------------------------------------------------- host math
def _build_mats(W_prob, W_unc, W_total):
    a0 = W_total[:, 0] * W_prob[:, 0]
    a1 = W_total[:, 0] * W_prob[:, 1]
    s = a0 + a1
    Wc = W_total[:, 1][:, None, None, None] * W_unc      # [C,2,3,3]

    # ps = b48.T @ vt + dmat_s.T @ rgb + dmat_a.T @ diff  ==  Q - 1
    b48 = np.zeros((K, M), np.float32)
    cm = np.arange(C) * YL
    for kx in range(3):
        for i in range(2):
            for r in range(8):
                for ky in range(3):
                    yl = r - ky
                    if 0 <= yl < YL:
                        b48[kx * 16 + i * 8 + r, cm + yl] = -Wc[:, i, ky, kx]

    m = np.arange(M)
    dmat_s = np.zeros((M, M), np.float32)
    dmat_s[m, m] = -s[m // YL]
    dmat_a = np.zeros((M, M), np.float32)
    dmat_a[m, m] = a1[m // YL]
    return b48.astype(NPBF), dmat_s.astype(NPBF), dmat_a.astype(NPBF)


def _pack_groups(x):
    """[B, C, H, W] f32 -> [NGRP, M, W] bf16 row-group blocks (m = 6c+yl)."""
    p = np.zeros((B, C, GPI * YL, W), NPBF)
    p[:, :, :H] = x.astype(NPBF)
    return np.ascontiguousarray(
        p.reshape(B, C, GPI, YL, W).transpose(0, 2, 1, 3, 4)
        .reshape(NGRP, M, W))


def _pack_vars(rgb_var, d_var):
    """[NGRP, 16, VW] bf16: partition i*8+r = map i, halo row 6gl-1+r."""
    vpad = np.zeros((B, 2, GPI * YL + 6, W + 2), np.float32)
    vpad[:, 0, 1:H + 1, 1:W + 1] = rgb_var[:, 0]
    vpad[:, 1, 1:H + 1, 1:W + 1] = d_var[:, 0]
    win = np.lib.stride_tricks.sliding_window_view(vpad, 8, axis=2)
    v = win[:, :, ::YL]                        # [B, 2, 86, W+2, 8]
    v = v.transpose(0, 2, 1, 4, 3).reshape(NGRP, 16, W + 2)
    out = np.zeros((NGRP, 16, VW), NPBF)
    out[:, :, :W + 2] = v.astype(NPBF)
    return out


def _iter_blocks(slab, ng_list):
    """[NG, P, W] -> list of [P, ng*W] per-iter blocks (groups side by side)."""
    blocks = []
    for j0, ng in ng_list:
        blocks.append(np.ascontiguousarray(
            slab[j0:j0 + ng].transpose(1, 0, 2).reshape(slab.shape[1], -1)))
    return blocks


# ------------------------------------------------------------- bass program
_CACHE = {}


def _build_program():
    nc = bacc.Bacc("TRN2", debug=False, num_devices=NCORES)
    rd_a = nc.dram_tensor("rd_a", [10, M, 8192], BF16, kind="ExternalInput").ap()
    rd_b = nc.dram_tensor("rd_b", [M, 6144], BF16, kind="ExternalInput").ap()
    var_a = nc.dram_tensor("var_a", [128, 4 * VW], BF16, kind="ExternalInput").ap()
    var_b = nc.dram_tensor("var_b", [48, 4 * VW], BF16, kind="ExternalInput").ap()
    b48_d = nc.dram_tensor("b48", [K, M], BF16, kind="ExternalInput").ap()
    dms_d = nc.dram_tensor("dmat_s", [M, M], BF16, kind="ExternalInput").ap()
    dma_d = nc.dram_tensor("dmat_a", [M, M], BF16, kind="ExternalInput").ap()
    out_a = nc.dram_tensor("out_a", [10, M, 4096], BF16, kind="ExternalOutput").ap()
    out_b = nc.dram_tensor("out_b", [M, 3072], BF16, kind="ExternalOutput").ap()

    VB = 4 * VW                                  # 4128 var cols per iter block
    with tile.TileContext(nc) as tc:
        with (
            tc.tile_pool(name="wpool", bufs=1) as wpool,
            tc.tile_pool(name="io", bufs=3) as io,
            tc.tile_pool(name="vtp", bufs=2) as vtp,
            tc.tile_pool(name="tqp", bufs=2) as tqp,
            tc.tile_pool(name="otp", bufs=2) as otp,
            tc.tile_pool(name="psum", bufs=2, space="PSUM") as psum,
        ):
            b48_sb = wpool.tile([K, M], BF16, name="b48_sb")
            nc.gpsimd.dma_start(out=b48_sb[:], in_=b48_d[:])
            dms_sb = wpool.tile([M, M], BF16, name="dms_sb")
            nc.gpsimd.dma_start(out=dms_sb[:], in_=dms_d[:])
            dma_sb = wpool.tile([M, M], BF16, name="dma_sb")
            nc.gpsimd.dma_start(out=dma_sb[:], in_=dma_d[:])
            var_res = wpool.tile([128, 2 * VB], BF16, name="var_res")
            nc.gpsimd.dma_start(out=var_res[:, :VB], in_=var_a[:])
            nc.gpsimd.dma_start(out=var_res[:48, VB:], in_=var_b[:])

            for it, (j0, ng) in enumerate(ITERS):
                Wi = ng * 1024
                rdt = io.tile([M, 8192], BF16, tag="rd", name=f"rd{it}")
                src = rd_a[it] if ng == 4 else rd_b[:]
                nc.sync.dma_start(out=rdt[:, :2 * Wi], in_=src)
                rt = rdt[:, 0:Wi]
                ft = rdt[:, Wi:2 * Wi]

                # build the 3 kx-shifted conv operands on-chip (SBUF->SBUF)
                st, cb = it % 8, (it // 8) * VB
                vt = vtp.tile([K, 4096], BF16, tag="vt", name=f"vt{it}")
                vsrc = var_res[16 * st:16 * st + 16, cb:cb + ng * VW]
                vsrc = vsrc.rearrange("p (g x) -> p g x", x=VW)
                for kx in range(3):
                    nc.gpsimd.dma_start(
                        out=vt[16 * kx:16 * kx + 16, :Wi].rearrange(
                            "p (g x) -> p g x", x=1024),
                        in_=vsrc[:, :, kx:kx + 1024])

                tq = tqp.tile([M, 4096], BF16, tag="tq", name=f"tq{it}")
                for h0 in range(0, Wi, 2048):
                    hw = min(2048, Wi - h0)
                    ps = psum.tile([M, 2048], F32, tag="ps", name=f"ps{it}_{h0}")
                    for xb in range(0, hw, 512):
                        nc.tensor.matmul(
                            ps[:, xb:xb + 512], b48_sb[:, :],
                            vt[:, h0 + xb:h0 + xb + 512],
                            start=True, stop=False)
                    for xb in range(0, hw, 512):
                        nc.tensor.matmul(
                            ps[:, xb:xb + 512], dms_sb[:, :],
                            rt[:, h0 + xb:h0 + xb + 512],
                            start=False, stop=False)
                    for xb in range(0, hw, 512):
                        nc.tensor.matmul(
                            ps[:, xb:xb + 512], dma_sb[:, :],
                            ft[:, h0 + xb:h0 + xb + 512],
                            start=False, stop=True)
                    # tq = ps + 1 = Q
                    nc.scalar.activation(
                        tq[:, h0:h0 + hw], ps[:, :hw],
                        mybir.ActivationFunctionType.Copy, bias=1.0)

                pt = otp.tile([M, 4096], BF16, tag="pt", name=f"pt{it}")
                nc.vector.tensor_mul(out=pt[:, :Wi], in0=ft, in1=tq[:, :Wi])
                ot = otp.tile([M, 4096], BF16, tag="ot", name=f"ot{it}")
                nc.vector.tensor_sub(out=ot[:, :Wi], in0=rt, in1=pt[:, :Wi])
                dst = out_a[it] if ng == 4 else out_b[:]
                nc.scalar.dma_start(out=dst, in_=ot[:, :Wi])

    nc.compile()
    return nc


def _shard_inputs(rgb, d, rgb_var, d_var, W_prob, W_unc, W_total):
    rgb = np.asarray(rgb, np.float32)
    d = np.asarray(d, np.float32)
    b48, dmat_s, dmat_a = _build_mats(
        np.asarray(W_prob, np.float32),
        np.asarray(W_unc, np.float32),
        np.asarray(W_total, np.float32))

    pg_r = _pack_groups(rgb)
    pg_f = _pack_groups(rgb - d)
    pg_v = _pack_vars(np.asarray(rgb_var, np.float32),
                      np.asarray(d_var, np.float32))

    in_maps = []
    for core in range(NCORES):
        sl = slice(core * NG, (core + 1) * NG)
        br = _iter_blocks(pg_r[sl], ITERS)
        bf = _iter_blocks(pg_f[sl], ITERS)
        rd_aa = np.concatenate(
            [np.concatenate([br[i], bf[i]], axis=1)[None] for i in range(10)])
        rd_bb = np.concatenate([br[10], bf[10]], axis=1)

        vcore = pg_v[sl]                                   # [43, 16, VW]
        va = np.ascontiguousarray(
            vcore[:32].reshape(8, 4, 16, VW).transpose(0, 2, 1, 3)
            .reshape(128, 4 * VW))
        vb12 = np.zeros((12, 16, VW), NPBF)
        vb12[:11] = vcore[32:]
        vb = np.ascontiguousarray(
            vb12.reshape(3, 4, 16, VW).transpose(0, 2, 1, 3)
            .reshape(48, 4 * VW))

        in_maps.append({
            "rd_a": rd_aa, "rd_b": rd_bb, "var_a": va, "var_b": vb,
            "b48": b48, "dmat_s": dmat_s, "dmat_a": dmat_a,
        })
    return in_maps


def _unshard_output(results):
    og = np.empty((NGRP, M, W), NPBF)
    for core in range(NCORES):
        oa = np.asarray(results[core]["out_a"], NPBF)
        ob = np.asarray(results[core]["out_b"], NPBF)
        g0 = core * NG
        og[g0:g0 + 40] = (
            oa.reshape(10, M, 4, W).transpose(0, 2, 1, 3).reshape(40, M, W))
        og[g0 + 40:g0 + 43] = (
            ob.reshape(M, 3, W).transpose(1, 0, 2))
    out = (og.reshape(B, GPI, C, YL, W).transpose(0, 2, 1, 3, 4)
           .reshape(B, C, GPI * YL, W)[:, :, :H].astype(np.float32))
    return np.ascontiguousarray(out)


def run(trace=False, **inputs):
    if "nc" not in _CACHE:
        _CACHE["nc"] = _build_program()
    nc = _CACHE["nc"]
    in_maps = _shard_inputs(**inputs)
    res = run_bass_kernel_spmd(nc, in_maps, list(range(NCORES)), trace=trace)
    return _unshard_output(res.results), res


def kernel(**inputs):
    out, _ = run(trace=False, **inputs)
    return out
